# revision 100
# baseline (speedup 1.0000x reference)
"""Trainium2 Bass kernel for nn_MultiHeadAttn (dense transformer block).

Strategy: data-parallel over batch (16 batches -> 8 cores x 2).
Per core, per batch element:
  xT = transpose(x) on PE;  q/k/v = xT.T @ W*T (bf16 matmuls);
  RoPE on q/k in natural [s, hd] layout with host-side even/odd weight-row
  permutation + duplicated cos/sin coefficient tensors (2 DVE muls + 2 Pool
  adds per half-block);
  qT/kT via PE transpose;  scoresT[j,i] = kT_h.T @ qT_h per head (K=64);
  E = exp(scoresT/8 + mask_bias) on ACT (softmax max-subtraction skipped --
  scores are tiny by construction; mathematically identical result);
  PV'[0:65] = [v_h | 1].T @ E  (row 64 = softmax denominator);
  postxT_h = PV' * 1/rowsum;  out = postxT.T @ WoT + residual -> DeepNorm LN
  (rstd = exp(-0.5*ln(var+eps)) so ACT stays on the exp/ln table set).

Zero biases / zero beta / unit gamma detected host-side are compiled out
(flags key the program cache; nonzero values take the general path).

Weights are transposed/permuted host-side (pure layout marshalling); all
FLOPs run on-device.
"""

import os
import sys

import numpy as np
import ml_dtypes

for _p in ("/opt/trn_rl_repo", "/root/.axon_site/_ro/trn_rl_repo"):
    if os.path.isdir(_p) and _p not in sys.path:
        sys.path.insert(0, _p)

import concourse.bass as bass
import concourse.tile as tile
from concourse import bacc, mybir
from concourse.bass_utils import run_bass_kernel_spmd

F32 = mybir.dt.float32
F32R = mybir.dt.float32r
BF16 = mybir.dt.bfloat16
FP8 = mybir.dt.float8e4
AF = mybir.ActivationFunctionType

B, S, HID = 16, 512, 512
NH, D = 14, 64
NHD = NH * D          # 896
P = 128
NCORES = 8
BPC = B // NCORES     # 2 batch elements per core
MCH = S // P          # 4 s-chunks
KCH = HID // P        # 4 hid-chunks
CCH = NHD // P        # 7 hd-chunks
HALF = NHD // 2       # 448
H7 = 7                # heads per half
D2 = D // 2           # 32
VW = 912              # padded v-pair row: NH*65=910 -> %16 == 0
LN_EPS = 1e-5
MASK_NEG = -30.0


def _emit(ctx, tc, repeat=1, flags=(True, True, True, True)):
    zb_qkv, zb_o, z_beta, ones_gamma = flags
    nc = tc.nc
    # xbf16 is host-pretiled to [BPC, P, MCH, HID] so each partition's
    # DMA read is one contiguous run (f32 x is not needed on device: the
    # residual is folded into the Wo PSUM via an identity matmul on xbf)
    xbfd = nc.dram_tensor("xbf16", [BPC, P, MCH, HID], BF16,
                          kind="ExternalInput")
    # duplicated-half rope coefficients: [cos|cos] and [sin|sin] per head
    cqcd = nc.dram_tensor("cqc", [BPC, S, NHD], BF16, kind="ExternalInput")
    cqsd = nc.dram_tensor("cqs", [BPC, S, NHD], BF16, kind="ExternalInput")
    ckcd = nc.dram_tensor("ckc", [BPC, S, NHD], BF16, kind="ExternalInput")
    cksd = nc.dram_tensor("cks", [BPC, S, NHD], BF16, kind="ExternalInput")
    mbd = nc.dram_tensor("maskb", [BPC, S, 1], F32, kind="ExternalInput")
    # weights host-pretiled to [2(half), P, KCH, HALF] so each half is a
    # fully contiguous per-partition DMA (first proj gates on half 0 only)
    wqtd = nc.dram_tensor("wqT", [P, KCH, NHD], BF16, kind="ExternalInput")
    wktd = nc.dram_tensor("wkT", [P, KCH, NHD], BF16, kind="ExternalInput")
    wvtd = nc.dram_tensor("wvT", [P, KCH, NHD], BF16, kind="ExternalInput")
    wotd = nc.dram_tensor("woT", [P, CCH, HID], BF16, kind="ExternalInput")
    bqd = nc.dram_tensor("bq", [1, NHD], BF16, kind="ExternalInput")
    bkd = nc.dram_tensor("bk", [1, NHD], BF16, kind="ExternalInput")
    bvd = nc.dram_tensor("bv", [1, NHD], BF16, kind="ExternalInput")
    bod = nc.dram_tensor("bo", [1, HID], F32R, kind="ExternalInput")
    gammad = nc.dram_tensor("gamma", [1, HID], F32, kind="ExternalInput")
    betad = nc.dram_tensor("beta", [1, HID], F32, kind="ExternalInput")
    identd = nc.dram_tensor("ident", [P, P], BF16, kind="ExternalInput")
    yd = nc.dram_tensor("y", [BPC, S, HID], F32, kind="ExternalOutput")

    # ---- pools ----
    consts = ctx.enter_context(tc.tile_pool(name="consts", bufs=1))
    bigp = ctx.enter_context(tc.tile_pool(name="big", bufs=1))  # per-batch persistents
    cfp = ctx.enter_context(tc.tile_pool(name="cf", bufs=8))  # coef tiles
    qrp = ctx.enter_context(tc.tile_pool(name="qr", bufs=8))  # post-rope proj
    rtp = ctx.enter_context(tc.tile_pool(name="rt", bufs=12))  # rope temps
    vp = ctx.enter_context(tc.tile_pool(name="vp", bufs=8))   # v tiles (4 j-chunks)
    etp = ctx.enter_context(tc.tile_pool(name="et", bufs=16))  # exp(scoresT) tiles
    rsp = ctx.enter_context(tc.tile_pool(name="rs", bufs=6))  # recip rows
    rbp = ctx.enter_context(tc.tile_pool(name="rb", bufs=6))  # broadcast recip
    lnp = ctx.enter_context(tc.tile_pool(name="ln", bufs=3))  # layernorm temps
    mbp = ctx.enter_context(tc.tile_pool(name="mb", bufs=8))  # mask bias tiles

    mmps = ctx.enter_context(tc.tile_pool(name="mmps", bufs=4, space="PSUM"))
    tpps = ctx.enter_context(tc.tile_pool(name="tpps", bufs=2, space="PSUM"))
    pvps = ctx.enter_context(tc.tile_pool(name="pvps", bufs=2, space="PSUM"))

    # ---- helpers ----
    def load_masks(b, i):
        mbs = []
        for jm in range(MCH):
            t = mbp.tile([P, 1], F32, tag="mb", name=f"mb_{i}_{jm}")
            nc.sync.dma_start(t, mbd[b, jm * P:(jm + 1) * P, :])
            mbs.append(t)
        return mbs

    batches = [b for _ in range(repeat) for b in range(BPC)]
    ST = {}

    def prep_thunks(i, b):
        """Emit-slices building xT, qT, kT, v for (i, b)."""
        st = {}
        ST[i] = st

        def t_load():
            # m-chunked so the first transposes start after 1/4 of the load
            st["xbf"] = []
            for m in range(MCH):
                t = bigp.tile([P, HID], BF16, tag="xbf", bufs=8,
                              name=f"xbf_{i}_{m}")
                (nc.sync if i == 0 else nc.gpsimd).dma_start(
                    t, xbfd[b][:, m, :])
                st["xbf"].append(t)
            st["mb_t"] = load_masks(b, i)
            st["xT"] = bigp.tile([P, KCH, S], BF16, tag="xT", bufs=2,
                                 name=f"xT_{i}")
            st["qT"] = bigp.tile([P, CCH, S], BF16, tag="qT", bufs=2,
                                 name=f"qT_{i}")
            st["kT"] = bigp.tile([P, CCH, S], BF16, tag="kT", bufs=2,
                                 name=f"kT_{i}")
            # v in fp8 jm-pairs [P, 2, VW] for DoubleRow PV matmuls.
            # VW pads NH*65=910 to a multiple of 16: the dual-fp8 ldweights
            # ISA restriction requires the pair stride % 16 == 0
            st["v"] = [vp.tile([P, 2, VW], FP8, tag="v",
                               name=f"v_{i}_{jp}")
                       for jp in range(MCH // 2)]
            for jp in range(MCH // 2):
                for tw in range(2):
                    nc.vector.memset(
                        st["v"][jp][:, tw, 0:NH * 65]
                        .rearrange("p (h e) -> p h e", e=65)[:, :, 64:65],
                        1.0)
            st["prs"] = {"q": [None] * MCH, "k": [None] * MCH}

        yield t_load

        def t_xt_m(m):
            ps = tpps.tile([P, KCH, P], BF16, tag="tp")
            for k in range(KCH):
                nc.tensor.transpose(ps[:, k, :],
                                    st["xbf"][m][:, k * P:(k + 1) * P],
                                    identb)
            nc.any.tensor_copy(st["xT"][:, :, m * P:(m + 1) * P], ps)

        for m in range(MCH):
            yield (lambda m=m: t_xt_m(m))

        def t_coef(m):
            for nm, dram in (("cqc", cqcd), ("cqs", cqsd),
                             ("ckc", ckcd), ("cks", cksd)):
                t = cfp.tile([P, NHD], BF16, tag="cf", name=f"{nm}_{i}_{m}")
                nc.sync.dma_start(
                    t, dram[b].rearrange("(m p) n -> p m n", p=P)[:, m, :])
                st[f"{nm}_{m}"] = t

        def t_proj(m, half):
            """q/k/v projections for one half of the head dim, sharing
            lhsT across the three weight matrices, then rope on q/k and
            the PSUM->SBUF copy for v."""
            hsl = slice(half * HALF, (half + 1) * HALF)
            pss = {}
            for w in ("q", "k", "v"):
                pss[w] = mmps.tile([P, HALF], F32, tag="mm",
                                   name=f"ps{w}_{i}_{m}_{half}")
            wmap = {"q": wqT, "k": wkT, "v": wvT}
            for k in range(KCH):
                for w in ("q", "k", "v"):
                    nc.tensor.matmul(
                        pss[w],
                        lhsT=st["xT"][:, k, m * P:(m + 1) * P],
                        rhs=wmap[w][:, k, hsl],
                        start=(k == 0),
                        stop=(zb_qkv and k == KCH - 1),
                    )
            if not zb_qkv:
                for w in ("q", "k", "v"):
                    bias_sb = {"q": bq_sb, "k": bk_sb, "v": bv_sb}[w]
                    nc.tensor.matmul(pss[w], lhsT=ones1, rhs=bias_sb[:, hsl],
                                     start=False, stop=True)
            # v: straight copy into the fp8 [P, 2, VW] pair tile
            # (alternate ACT/DVE to balance engine load)
            vdst = (st["v"][m // 2][:, m % 2,
                                    half * H7 * 65:(half + 1) * H7 * 65]
                    .rearrange("p (h e) -> p h e", e=65)[:, :, 0:64])
            vsrc = pss["v"].rearrange("p (h d) -> p h d", d=D)
            if (m + half) % 2 == 0:
                nc.scalar.copy(vdst, vsrc)
            else:
                nc.vector.tensor_copy(vdst, vsrc)
            # rope on q/k: duplicated-coef form.
            #   proj layout per head (permuted weights): [x0(32) | x1(32)]
            #   cc = proj * [cos|cos]; ss = proj * [sin|sin]
            #   out[0:32] = cc[0:32] - ss[32:64]; out[32:64] = cc[32:64] + ss[0:32]
            # PSUM is read once (copy to packed bf16 SBUF) so both muls
            # run in the DVE 2x/4x perf modes.
            for w in ("q", "k"):
                if st["prs"][w][m] is None:
                    pr = qrp.tile([P, NHD], BF16, tag="qr",
                                  name=f"pr_{i}_{m}_{w}")
                    st["prs"][w][m] = pr
                pr = st["prs"][w][m]
                ccoef = st[f"c{w}c_{m}"]
                scoef = st[f"c{w}s_{m}"]
                qsb = rtp.tile([P, HALF], BF16, tag="rt")
                if w == "q":
                    nc.scalar.copy(qsb, pss[w])
                else:
                    nc.vector.tensor_copy(qsb, pss[w])
                cc = rtp.tile([P, HALF], BF16, tag="rt")
                ss = rtp.tile([P, HALF], BF16, tag="rt")
                nc.vector.tensor_mul(cc, qsb, ccoef[:, hsl])
                nc.vector.tensor_mul(ss, qsb, scoef[:, hsl])
                pr3 = pr[:, hsl].rearrange("p (h d) -> p h d", d=D)
                cc3 = cc.rearrange("p (h d) -> p h d", d=D)
                ss3 = ss.rearrange("p (h d) -> p h d", d=D)
                nc.gpsimd.tensor_sub(pr3[:, :, 0:D2],
                                     cc3[:, :, 0:D2], ss3[:, :, D2:D])
                nc.gpsimd.tensor_add(pr3[:, :, D2:D],
                                     cc3[:, :, D2:D], ss3[:, :, 0:D2])

        for m in range(MCH):
            yield (lambda m=m: t_coef(m))
            for half in range(2):
                yield (lambda m=m, h=half: t_proj(m, h))

        def t_qkt(which, c):
            dstT = st["qT"] if which == "q" else st["kT"]
            ps = tpps.tile([P, S], BF16, tag="tp")
            for m in range(MCH):
                nc.tensor.transpose(ps[:, m * P:(m + 1) * P],
                                    st["prs"][which][m][:, c * P:(c + 1) * P],
                                    identb)
            nc.any.tensor_copy(dstT[:, c, :], ps)

        for c in range(CCH):
            yield (lambda c=c: t_qkt("q", c))
            yield (lambda c=c: t_qkt("k", c))

    def att_core_thunks(i, b):
        st = ST[i]

        def t_alloc():
            st["postxT"] = bigp.tile([P, CCH, S], BF16, tag="postxT",
                                     bufs=2, name=f"px_{i}")

        yield t_alloc

        e_tiles = {}

        def t_scores(hc):
            qT_sb, kT_sb, mb_t = st["qT"], st["kT"], st["mb_t"]
            # e tiles in fp8 jm-pairs [P, 2, S] for DoubleRow PV matmuls
            e_tiles[hc] = ([None, None], [None, None])
            for jm in range(MCH):
                for par in (0, 1):
                    h = 2 * hc + par
                    rows = slice(par * 64, par * 64 + 64)
                    ps_s = mmps.tile([P, S], F32, tag="mm",
                                     name=f"ss_{i}_{h}_{jm}")
                    nc.tensor.matmul(
                        ps_s,
                        lhsT=kT_sb[rows, hc, jm * P:(jm + 1) * P],
                        rhs=qT_sb[rows, hc, :],
                    )
                    if jm % 2 == 0:
                        e_tiles[hc][par][jm // 2] = etp.tile(
                            [P, 2, S], FP8, tag="et",
                            name=f"et_{i}_{h}_{jm // 2}")
                    e_t = e_tiles[hc][par][jm // 2]
                    nc.scalar.activation(e_t[:, jm % 2, :], ps_s, AF.Exp,
                                         bias=mb_t[jm], scale=1.0 / np.sqrt(D))

        def t_pv(hc):
            v_sb, postxT = st["v"], st["postxT"]
            for par in (0, 1):
                h = 2 * hc + par
                ps_pv = pvps.tile([P, S], F32, tag="pv", name=f"pv_{i}_{h}")
                for jp in range(MCH // 2):
                    nc.tensor.matmul(
                        ps_pv[0:65, :],
                        lhsT=v_sb[jp][:, :, h * 65:h * 65 + 65],
                        rhs=e_tiles[hc][par][jp][:],
                        start=(jp == 0),
                        stop=(jp == MCH // 2 - 1),
                        perf_mode=mybir.MatmulPerfMode.DoubleRow,
                    )
                rr_t = rsp.tile([1, S], F32, tag="rr")
                nc.vector.reciprocal(rr_t, ps_pv[64:65, :])
                rb_t = rbp.tile([64, S], F32, tag="rb")
                nc.gpsimd.partition_broadcast(rb_t, rr_t)
                dst = (postxT[0:64, hc, :] if par == 0
                       else postxT[64:128, hc, :])
                nc.vector.tensor_mul(dst, ps_pv[0:64, :], rb_t)
            del e_tiles[hc]

        # software pipeline: scores(hc+1) issues before pv(hc) so the PE
        # never waits on the exp chain of the head it just scored
        yield (lambda: t_scores(0))
        for hc in range(CCH):
            if hc + 1 < CCH:
                yield (lambda hc=hc: t_scores(hc + 1))
            yield (lambda hc=hc: t_pv(hc))

    def out_thunks(i, b, last=False):
        st = ST[i]

        def t_out_a(m):
            ps_o = mmps.tile([P, HID], F32, tag="mm")
            for c in range(CCH):
                nc.tensor.matmul(
                    ps_o,
                    lhsT=st["postxT"][:, c, m * P:(m + 1) * P],
                    rhs=woT[:, c, :],
                    start=(c == 0),
                    stop=False,
                )
            if not zb_o:
                nc.tensor.matmul(ps_o, lhsT=ones1r, rhs=bo_row,
                                 start=False, stop=False)
            # residual fold: ps_o += I.T @ xbf[m]  (adds x in one matmul)
            nc.tensor.matmul(ps_o, lhsT=identb, rhs=st["xbf"][m],
                             start=False, stop=True)
            y_t = lnp.tile([P, HID], F32, tag="y", bufs=4,
                           name=f"y_{i}_{m}")
            nc.vector.tensor_copy(y_t, ps_o)
            st_t = lnp.tile([P, 6], F32, tag="st")
            nc.vector.bn_stats(st_t, y_t)
            nc.vector.bn_aggr(st["mv4"][:, m, :], st_t)
            st["y_t"][m] = y_t

        def t_ln_scale(msl, part):
            # one Sqrt per batch (batched over the 4 s-chunks) so the ACT
            # table only round-trips exp->sqrt->exp once per batch element.
            # (the final batch splits in two so its tail overlaps better)
            mv4 = st["mv4"]
            nmch = msl.stop - msl.start
            sd4 = lnp.tile([P, nmch], F32, tag="sd4")
            nc.scalar.activation(sd4, mv4[:, msl, 1], AF.Sqrt, bias=eps_t)
            rstd4 = lnp.tile([P, nmch], F32, tag="rstd4", bufs=4,
                             name=f"rstd4_{i}_{part}")
            nc.vector.reciprocal(rstd4, sd4)
            # nmurs = -mu * rstd (per-partition bias for the affine tail)
            nmurs4 = lnp.tile([P, nmch], F32, tag="nmurs4", bufs=4,
                              name=f"nmurs4_{i}_{part}")
            nc.vector.scalar_tensor_tensor(nmurs4, mv4[:, msl, 0], -1.0,
                                           rstd4,
                                           op0=mybir.AluOpType.mult,
                                           op1=mybir.AluOpType.mult)
            for m in range(msl.start, msl.stop):
                st["rstd"][m] = rstd4[:, m - msl.start:m - msl.start + 1]
                st["nmurs"][m] = nmurs4[:, m - msl.start:m - msl.start + 1]

        def t_out_b(m):
            yb = yd[b].rearrange("(m p) h -> p m h", p=P)
            y_t = st["y_t"][m]
            y2 = lnp.tile([P, HID], F32, tag="y2", bufs=4)
            if ones_gamma:
                # y2 = (y_t * rstd + (-mu*rstd)) + 0
                nc.vector.affine_then_add(y2, y_t, zeros_sb,
                                          scale=st["rstd"][m],
                                          bias=st["nmurs"][m])
            else:
                nc.vector.scalar_tensor_tensor(y2, y_t, st["mv4"][:, m, 0:1],
                                               gamma_sb,
                                               op0=mybir.AluOpType.subtract,
                                               op1=mybir.AluOpType.mult)
                nc.vector.tensor_scalar_mul(y2, y2, st["rstd"][m])
            if not z_beta:
                nc.gpsimd.tensor_add(y2, y2, beta_sb)
            nc.sync.dma_start(yb[:, m, :], y2)

        def t_alloc_mv():
            st["mv4"] = lnp.tile([P, MCH, 2], F32, tag="mv4", bufs=2,
                                 name=f"mv4_{i}")
            st["y_t"] = [None] * MCH
            st["rstd"] = [None] * MCH
            st["nmurs"] = [None] * MCH

        yield t_alloc_mv
        if last:
            # per-m LN groups: out_a(m+1) overlaps affine(m)+DMA(m); no
            # extra table loads since no exps follow the final batch
            for m in range(MCH):
                yield (lambda m=m: t_out_a(m))
                yield (lambda m=m: t_ln_scale(slice(m, m + 1), m))
                yield (lambda m=m: t_out_b(m))
        else:
            for m in range(MCH):
                yield (lambda m=m: t_out_a(m))
            yield (lambda: t_ln_scale(slice(0, MCH), 0))
            for m in range(MCH):
                yield (lambda m=m: t_out_b(m))

    # ---- prologue: ident first (tiny, gates transposes), then batch-0
    # xbf/masks, then weights (first proj waits on them) ----
    identb = consts.tile([P, P], BF16, tag="identb")
    nc.sync.dma_start(identb, identd[:])
    pre0 = list(prep_thunks(0, batches[0]))
    pre0[0]()                                   # xbf/mask DMAs
    wqT = consts.tile([P, KCH, NHD], BF16, tag="wqT")
    nc.sync.dma_start(wqT, wqtd[:])
    wkT = consts.tile([P, KCH, NHD], BF16, tag="wkT")
    nc.sync.dma_start(wkT, wktd[:])
    wvT = consts.tile([P, KCH, NHD], BF16, tag="wvT")
    nc.sync.dma_start(wvT, wvtd[:])
    if not zb_qkv:
        bq_sb = consts.tile([1, NHD], BF16, tag="bq")
        nc.sync.dma_start(bq_sb, bqd[:])
        bk_sb = consts.tile([1, NHD], BF16, tag="bk")
        nc.sync.dma_start(bk_sb, bkd[:])
        bv_sb = consts.tile([1, NHD], BF16, tag="bv")
        nc.sync.dma_start(bv_sb, bvd[:])
        ones1 = consts.tile([1, P], BF16, tag="ones1")
        nc.vector.memset(ones1, 1.0)
    if not zb_o:
        bo_row = consts.tile([1, HID], F32R, tag="bo_row")
        nc.sync.dma_start(bo_row, bod[:])
        ones1r = consts.tile([1, P], F32R, tag="ones1r")
        nc.vector.memset(ones1r.bitcast(F32), 1.0)
    eps_t = consts.tile([P, 1], F32, tag="eps")
    nc.vector.memset(eps_t, LN_EPS)
    if ones_gamma:
        zeros_sb = consts.tile([P, HID], F32, tag="zeros")
        nc.vector.memset(zeros_sb, 0.0)
    for t in pre0[1:]:
        t()
    woT = consts.tile([P, CCH, HID], BF16, tag="woT")
    nc.sync.dma_start(woT, wotd[:])
    if not ones_gamma:
        gamma_sb = consts.tile([P, HID], F32, tag="gamma")
        nc.sync.dma_start(gamma_sb, gammad[:].to_broadcast([P, HID]))
    if not z_beta:
        beta_sb = consts.tile([P, HID], F32, tag="beta")
        nc.sync.dma_start(beta_sb, betad[:].to_broadcast([P, HID]))

    # ---- steady state ----
    # batch i's attention runs interleaved with batch i+1's prep AND
    # batch i-1's output/LN phase (fills PE bubbles during exp waits)
    def merge(a_list, b_list):
        """Proportionally interleave b_list into a_list."""
        na, nn = len(a_list), len(b_list)
        out = []
        ai = ni = 0
        while ai < na or ni < nn:
            if ai < na:
                out.append(a_list[ai])
                ai += 1
            take = (nn * (ai + 1)) // max(na, 1) - ni if na else nn
            for _ in range(max(0, take)):
                if ni < nn:
                    out.append(b_list[ni])
                    ni += 1
        return out

    prev_out = []
    for i, b in enumerate(batches):
        attc = list(att_core_thunks(i, b))
        stream_a = merge(attc, prev_out)
        nxt = (list(prep_thunks(i + 1, batches[i + 1]))
               if i + 1 < len(batches) else [])
        for t in merge(stream_a, nxt):
            t()
        prev_out = list(out_thunks(i, b, last=(i + 1 == len(batches))))
    for t in prev_out:
        t()


_NC = {}


def build(repeat=1, flags=(True, True, True, True)):
    key = (repeat, flags)
    if key in _NC:
        return _NC[key]
    nc = bacc.Bacc("TRN2", target_bir_lowering=False, debug=False,
                   enable_asserts=False, num_devices=NCORES)
    from contextlib import ExitStack
    with tile.TileContext(nc) as tc, ExitStack() as ctx:
        _emit(ctx, tc, repeat=repeat, flags=flags)
    nc.compile()
    _NC[key] = nc
    return nc


_PERM = np.concatenate([np.arange(0, D, 2), np.arange(1, D, 2)])  # evens|odds
_COLPERM = (np.arange(NH)[:, None] * D + _PERM[None, :]).reshape(-1)


def input_flags(inputs):
    zb_qkv = not (np.any(inputs["bq"]) or np.any(inputs["bk"])
                  or np.any(inputs["bv"]))
    zb_o = not np.any(inputs["bo"])
    z_beta = not np.any(inputs["ln_beta"])
    ones_gamma = bool(np.all(np.asarray(inputs["ln_gamma"]) == 1.0))
    return (zb_qkv, zb_o, z_beta, ones_gamma)


def make_in_maps(inputs):
    x = np.ascontiguousarray(np.asarray(inputs["x"], dtype=np.float32))
    # rope coefs: input layout per head is [sin(32) | cos(32)].
    # duplicate each half -> [cos|cos], [sin|sin]
    def coefs(name):
        c = np.asarray(inputs[name], np.float32)      # [B, S, NH, D]
        sin, cos = c[..., :D2], c[..., D2:]
        cdup = np.concatenate([cos, cos], axis=-1).reshape(B, S, NHD)
        sdup = np.concatenate([sin, sin], axis=-1).reshape(B, S, NHD)
        return (np.ascontiguousarray(cdup.astype(ml_dtypes.bfloat16)),
                np.ascontiguousarray(sdup.astype(ml_dtypes.bfloat16)))
    cqc, cqs = coefs("product_ion_info_query")
    ckc, cks = coefs("product_ion_info_key")
    mask = np.asarray(inputs["src_key_padding_mask"]).reshape(B, S)
    maskb = np.where(mask, 0.0, MASK_NEG).astype(np.float32).reshape(B, S, 1)
    def tile_w(w, nch):
        # [nch*P, free] -> [P, nch, free] (partition-contiguous DMA)
        return np.ascontiguousarray(
            w.reshape(nch, P, w.shape[-1]).transpose(1, 0, 2))

    def tile_w_half(w):
        # [KCH*P, NHD] -> [2, P, KCH, HALF], half-major contiguous
        t = tile_w(w, KCH)                        # [P, KCH, NHD]
        return np.ascontiguousarray(
            t.reshape(P, KCH, 2, HALF).transpose(2, 0, 1, 3))
    # permute q/k weight rows per head to [even components | odd components]
    wqT = tile_w(np.asarray(inputs["Wq"], np.float32).T[:, _COLPERM]
                 .astype(ml_dtypes.bfloat16), KCH)
    wkT = tile_w(np.asarray(inputs["Wk"], np.float32).T[:, _COLPERM]
                 .astype(ml_dtypes.bfloat16), KCH)
    wvT = tile_w(np.asarray(inputs["Wv"], np.float32).T
                 .astype(ml_dtypes.bfloat16), KCH)
    woT = tile_w(np.asarray(inputs["Wo"], np.float32).T
                 .astype(ml_dtypes.bfloat16), CCH)
    bq = np.asarray(inputs["bq"], np.float32)[_COLPERM].reshape(1, NHD)
    bk = np.asarray(inputs["bk"], np.float32)[_COLPERM].reshape(1, NHD)
    shared = dict(
        wqT=wqT, wkT=wkT, wvT=wvT, woT=woT,
        bq=bq.astype(ml_dtypes.bfloat16),
        bk=bk.astype(ml_dtypes.bfloat16),
        bv=np.asarray(inputs["bv"], np.float32).reshape(1, NHD).astype(ml_dtypes.bfloat16),
        bo=np.asarray(inputs["bo"], np.float32).reshape(1, HID),
        gamma=np.asarray(inputs["ln_gamma"], np.float32).reshape(1, HID),
        beta=np.asarray(inputs["ln_beta"], np.float32).reshape(1, HID),
        ident=np.eye(P, dtype=ml_dtypes.bfloat16),
    )
    # pre-tile x to [B, P, MCH, HID]; only the bf16 copy goes to device
    xbf16 = np.ascontiguousarray(
        x.reshape(B, MCH, P, HID).transpose(0, 2, 1, 3)
        .astype(ml_dtypes.bfloat16))
    in_maps = []
    for c in range(NCORES):
        sl = slice(c * BPC, (c + 1) * BPC)
        in_maps.append(dict(
            xbf16=np.ascontiguousarray(xbf16[sl]),
            cqc=np.ascontiguousarray(cqc[sl]),
            cqs=np.ascontiguousarray(cqs[sl]),
            ckc=np.ascontiguousarray(ckc[sl]),
            cks=np.ascontiguousarray(cks[sl]),
            maskb=np.ascontiguousarray(maskb[sl]),
            **shared,
        ))
    return in_maps


LAST_RESULTS = None
_RUNNERS = {}


def kernel(_repeat=1, **inputs):
    global LAST_RESULTS
    flags = input_flags(inputs)
    key = (_repeat, flags)
    if key not in _RUNNERS:
        _RUNNERS[key] = make_runner(_repeat, flags)
    in_maps = make_in_maps(inputs)
    out_arrs = _RUNNERS[key](in_maps)
    out = np.asarray(out_arrs[0]).reshape(NCORES, BPC, S, HID)
    return out.reshape(B, S, HID).astype(np.float32)


def make_runner(repeat=1, flags=(True, True, True, True)):
    """Build the sharded 8-core jit once; return f(in_maps)->list per-core outs.

    Mirrors bass2jax.run_bass_via_pjrt's multi-core path, but reusable so
    repeated calls skip retrace/recompile (for timing)."""
    import jax
    from jax.experimental.shard_map import shard_map
    from jax.sharding import Mesh, PartitionSpec
    from concourse import bass2jax

    nc = build(repeat, flags)
    bass2jax.install_neuronx_cc_hook()
    partition_name = (nc.partition_id_tensor.name
                      if nc.partition_id_tensor else None)
    in_names, out_names, out_avals, zero_outs = [], [], [], []
    for alloc in nc.m.functions[0].allocations:
        if not isinstance(alloc, mybir.MemoryLocationSet):
            continue
        name = alloc.memorylocations[0].name
        if alloc.kind == "ExternalInput":
            if name != partition_name:
                in_names.append(name)
        elif alloc.kind == "ExternalOutput":
            shape = tuple(alloc.tensor_shape)
            dtype = mybir.dt.np(alloc.dtype)
            out_names.append(name)
            out_avals.append(jax.core.ShapedArray(shape, dtype))
            zero_outs.append(np.zeros(shape, dtype))
    n_params = len(in_names)
    all_in_names = list(in_names) + list(out_names)
    if partition_name is not None:
        all_in_names.append(partition_name)

    def _body(*args):
        operands = list(args)
        if partition_name is not None:
            operands.append(bass2jax.partition_id_tensor())
        outs = bass2jax._bass_exec_p.bind(
            *operands,
            out_avals=tuple(out_avals),
            in_names=tuple(all_in_names),
            out_names=tuple(out_names),
            lowering_input_output_aliases=(),
            sim_require_finite=True,
            sim_require_nnan=True,
            nc=nc,
        )
        return tuple(outs)

    devices = jax.devices()[:NCORES]
    mesh = Mesh(np.asarray(devices), ("core",))
    n_outs = len(out_names)
    sharded = jax.jit(
        shard_map(_body, mesh=mesh,
                  in_specs=(PartitionSpec("core"),) * (n_params + n_outs),
                  out_specs=(PartitionSpec("core"),) * n_outs,
                  check_rep=False),
        keep_unused=True,
    )
    concat_zeros = [np.zeros((NCORES * z.shape[0], *z.shape[1:]), z.dtype)
                    for z in zero_outs]

    def run(in_maps):
        per_core = [[np.asarray(m[n]) for n in in_names] for m in in_maps]
        concat_in = [np.concatenate([per_core[c][i] for c in range(NCORES)],
                                    axis=0) for i in range(n_params)]
        out_arrs = sharded(*concat_in, *concat_zeros)
        jax.block_until_ready(out_arrs)
        return out_arrs

    return run


# revision 105
# speedup vs baseline: 1.0030x; 1.0030x over previous
"""Trainium2 Bass kernel for nn_MultiHeadAttn (dense transformer block).

Strategy: data-parallel over batch (16 batches -> 8 cores x 2).
Per core, per batch element:
  xT = transpose(x) on PE;  q/k/v = xT.T @ W*T (bf16 matmuls);
  RoPE on q/k in natural [s, hd] layout with host-side even/odd weight-row
  permutation + duplicated cos/sin coefficient tensors (2 DVE muls + 2 Pool
  adds per half-block);
  qT/kT via PE transpose;  scoresT[j,i] = kT_h.T @ qT_h per head (K=64);
  E = exp(scoresT/8 + mask_bias) on ACT (softmax max-subtraction skipped --
  scores are tiny by construction; mathematically identical result);
  PV'[0:65] = [v_h | 1].T @ E  (row 64 = softmax denominator);
  postxT_h = PV' * 1/rowsum;  out = postxT.T @ WoT + residual -> DeepNorm LN
  (rstd = exp(-0.5*ln(var+eps)) so ACT stays on the exp/ln table set).

Zero biases / zero beta / unit gamma detected host-side are compiled out
(flags key the program cache; nonzero values take the general path).

Weights are transposed/permuted host-side (pure layout marshalling); all
FLOPs run on-device.
"""

import os
import sys

import numpy as np
import ml_dtypes

for _p in ("/opt/trn_rl_repo", "/root/.axon_site/_ro/trn_rl_repo"):
    if os.path.isdir(_p) and _p not in sys.path:
        sys.path.insert(0, _p)

import concourse.bass as bass
import concourse.tile as tile
from concourse import bacc, mybir
from concourse.bass_utils import run_bass_kernel_spmd

F32 = mybir.dt.float32
F32R = mybir.dt.float32r
BF16 = mybir.dt.bfloat16
FP8 = mybir.dt.float8e4
AF = mybir.ActivationFunctionType

B, S, HID = 16, 512, 512
NH, D = 14, 64
NHD = NH * D          # 896
P = 128
NCORES = 8
BPC = B // NCORES     # 2 batch elements per core
MCH = S // P          # 4 s-chunks
KCH = HID // P        # 4 hid-chunks
CCH = NHD // P        # 7 hd-chunks
HALF = NHD // 2       # 448
H7 = 7                # heads per half
D2 = D // 2           # 32
VW = 912              # padded v-pair row: NH*65=910 -> %16 == 0
LN_EPS = 1e-5
MASK_NEG = -30.0


def _emit(ctx, tc, repeat=1, flags=(True, True, True, True)):
    zb_qkv, zb_o, z_beta, ones_gamma = flags
    nc = tc.nc
    # xbf16 is host-pretiled to [BPC, P, MCH, HID] so each partition's
    # DMA read is one contiguous run (f32 x is not needed on device: the
    # residual is folded into the Wo PSUM via an identity matmul on xbf)
    xbfd = nc.dram_tensor("xbf16", [BPC, P, MCH, HID], BF16,
                          kind="ExternalInput")
    # rope coefficients, one tensor per input: [cos(32)|sin(32)] per head.
    # the second rope product reads it through a half-swapped (negative
    # stride) view, so no duplicated copies are shipped
    cq1d = nc.dram_tensor("cq1", [BPC, S, NHD], BF16, kind="ExternalInput")
    ck1d = nc.dram_tensor("ck1", [BPC, S, NHD], BF16, kind="ExternalInput")
    mbd = nc.dram_tensor("maskb", [BPC, S, 1], F32, kind="ExternalInput")
    # weights host-pretiled to [2(half), P, KCH, HALF] so each half is a
    # fully contiguous per-partition DMA (first proj gates on half 0 only)
    wqtd = nc.dram_tensor("wqT", [P, KCH, NHD], BF16, kind="ExternalInput")
    wktd = nc.dram_tensor("wkT", [P, KCH, NHD], BF16, kind="ExternalInput")
    wvtd = nc.dram_tensor("wvT", [P, KCH, NHD], BF16, kind="ExternalInput")
    wotd = nc.dram_tensor("woT", [P, CCH, HID], BF16, kind="ExternalInput")
    bqd = nc.dram_tensor("bq", [1, NHD], BF16, kind="ExternalInput")
    bkd = nc.dram_tensor("bk", [1, NHD], BF16, kind="ExternalInput")
    bvd = nc.dram_tensor("bv", [1, NHD], BF16, kind="ExternalInput")
    bod = nc.dram_tensor("bo", [1, HID], F32R, kind="ExternalInput")
    gammad = nc.dram_tensor("gamma", [1, HID], F32, kind="ExternalInput")
    betad = nc.dram_tensor("beta", [1, HID], F32, kind="ExternalInput")
    identd = nc.dram_tensor("ident", [P, P], BF16, kind="ExternalInput")
    yd = nc.dram_tensor("y", [BPC, S, HID], F32, kind="ExternalOutput")

    # ---- pools ----
    consts = ctx.enter_context(tc.tile_pool(name="consts", bufs=1))
    bigp = ctx.enter_context(tc.tile_pool(name="big", bufs=1))  # per-batch persistents
    cfp = ctx.enter_context(tc.tile_pool(name="cf", bufs=8))  # coef tiles
    qrp = ctx.enter_context(tc.tile_pool(name="qr", bufs=8))  # post-rope proj
    rtp = ctx.enter_context(tc.tile_pool(name="rt", bufs=12))  # rope temps
    vp = ctx.enter_context(tc.tile_pool(name="vp", bufs=8))   # v tiles (4 j-chunks)
    etp = ctx.enter_context(tc.tile_pool(name="et", bufs=16))  # exp(scoresT) tiles
    rsp = ctx.enter_context(tc.tile_pool(name="rs", bufs=6))  # recip rows
    rbp = ctx.enter_context(tc.tile_pool(name="rb", bufs=6))  # broadcast recip
    lnp = ctx.enter_context(tc.tile_pool(name="ln", bufs=3))  # layernorm temps
    mbp = ctx.enter_context(tc.tile_pool(name="mb", bufs=8))  # mask bias tiles

    mmps = ctx.enter_context(tc.tile_pool(name="mmps", bufs=4, space="PSUM"))
    tpps = ctx.enter_context(tc.tile_pool(name="tpps", bufs=2, space="PSUM"))
    pvps = ctx.enter_context(tc.tile_pool(name="pvps", bufs=2, space="PSUM"))

    # ---- helpers ----
    def load_masks(b, i):
        mbs = []
        for jm in range(MCH):
            t = mbp.tile([P, 1], F32, tag="mb", name=f"mb_{i}_{jm}")
            nc.sync.dma_start(t, mbd[b, jm * P:(jm + 1) * P, :])
            mbs.append(t)
        return mbs

    batches = [b for _ in range(repeat) for b in range(BPC)]
    ST = {}

    def prep_thunks(i, b):
        """Emit-slices building xT, qT, kT, v for (i, b)."""
        st = {}
        ST[i] = st

        def t_load():
            # m-chunked so the first transposes start after 1/4 of the load
            st["xbf"] = []
            for m in range(MCH):
                t = bigp.tile([P, HID], BF16, tag="xbf", bufs=8,
                              name=f"xbf_{i}_{m}")
                (nc.sync if i == 0 else nc.gpsimd).dma_start(
                    t, xbfd[b][:, m, :])
                st["xbf"].append(t)
            st["mb_t"] = load_masks(b, i)
            st["xT"] = bigp.tile([P, KCH, S], BF16, tag="xT", bufs=2,
                                 name=f"xT_{i}")
            st["qT"] = bigp.tile([P, CCH, S], BF16, tag="qT", bufs=2,
                                 name=f"qT_{i}")
            st["kT"] = bigp.tile([P, CCH, S], BF16, tag="kT", bufs=2,
                                 name=f"kT_{i}")
            # v in fp8 jm-pairs [P, 2, VW] for DoubleRow PV matmuls.
            # VW pads NH*65=910 to a multiple of 16: the dual-fp8 ldweights
            # ISA restriction requires the pair stride % 16 == 0
            st["v"] = [vp.tile([P, 2, VW], FP8, tag="v",
                               name=f"v_{i}_{jp}")
                       for jp in range(MCH // 2)]
            for jp in range(MCH // 2):
                for tw in range(2):
                    nc.vector.memset(
                        st["v"][jp][:, tw, 0:NH * 65]
                        .rearrange("p (h e) -> p h e", e=65)[:, :, 64:65],
                        1.0)
            st["prs"] = {"q": [None] * MCH, "k": [None] * MCH}

        yield t_load

        def t_xt_m(m):
            ps = tpps.tile([P, KCH, P], BF16, tag="tp")
            for k in range(KCH):
                nc.tensor.transpose(ps[:, k, :],
                                    st["xbf"][m][:, k * P:(k + 1) * P],
                                    identb)
            nc.any.tensor_copy(st["xT"][:, :, m * P:(m + 1) * P], ps)

        for m in range(MCH):
            yield (lambda m=m: t_xt_m(m))

        def t_coef(m):
            for nm, dram in (("cq1", cq1d), ("ck1", ck1d)):
                t = cfp.tile([P, NHD], BF16, tag="cf", name=f"{nm}_{i}_{m}")
                nc.sync.dma_start(
                    t, dram[b].rearrange("(m p) n -> p m n", p=P)[:, m, :])
                st[f"{nm}_{m}"] = t

        def t_proj(m, half):
            """q/k/v projections for one half of the head dim, sharing
            lhsT across the three weight matrices, then rope on q/k and
            the PSUM->SBUF copy for v."""
            hsl = slice(half * HALF, (half + 1) * HALF)
            pss = {}
            for w in ("q", "k", "v"):
                pss[w] = mmps.tile([P, HALF], F32, tag="mm",
                                   name=f"ps{w}_{i}_{m}_{half}")
            wmap = {"q": wqT, "k": wkT, "v": wvT}
            for k in range(KCH):
                for w in ("q", "k", "v"):
                    nc.tensor.matmul(
                        pss[w],
                        lhsT=st["xT"][:, k, m * P:(m + 1) * P],
                        rhs=wmap[w][:, k, hsl],
                        start=(k == 0),
                        stop=(zb_qkv and k == KCH - 1),
                    )
            if not zb_qkv:
                for w in ("q", "k", "v"):
                    bias_sb = {"q": bq_sb, "k": bk_sb, "v": bv_sb}[w]
                    nc.tensor.matmul(pss[w], lhsT=ones1, rhs=bias_sb[:, hsl],
                                     start=False, stop=True)
            # v: straight copy into the fp8 [P, 2, VW] pair tile
            # (alternate ACT/DVE to balance engine load)
            vdst = (st["v"][m // 2][:, m % 2,
                                    half * H7 * 65:(half + 1) * H7 * 65]
                    .rearrange("p (h e) -> p h e", e=65)[:, :, 0:64])
            vsrc = pss["v"].rearrange("p (h d) -> p h d", d=D)
            if (m + half) % 2 == 0:
                nc.scalar.copy(vdst, vsrc)
            else:
                nc.vector.tensor_copy(vdst, vsrc)
            # rope on q/k: duplicated-coef form.
            #   proj layout per head (permuted weights): [x0(32) | x1(32)]
            #   cc = proj * [cos|cos]; ss = proj * [sin|sin]
            #   out[0:32] = cc[0:32] - ss[32:64]; out[32:64] = cc[32:64] + ss[0:32]
            # PSUM is read once (copy to packed bf16 SBUF) so both muls
            # run in the DVE 2x/4x perf modes.
            for w in ("q", "k"):
                if st["prs"][w][m] is None:
                    pr = qrp.tile([P, NHD], BF16, tag="qr",
                                  name=f"pr_{i}_{m}_{w}")
                    st["prs"][w][m] = pr
                pr = st["prs"][w][m]
                # coef layout per head: [cos(32) | sin(32)]
                c1 = st[f"c{w}1_{m}"][:, hsl]
                qsb = rtp.tile([P, HALF], BF16, tag="rt")
                if w == "q":
                    nc.scalar.copy(qsb, pss[w])
                else:
                    nc.vector.tensor_copy(qsb, pss[w])
                # m1 = [x0 | x1] * [cos | sin] = [x0*cos | x1*sin]
                # m2 = [x0 | x1] * [sin | cos] = [x0*sin | x1*cos]
                # (m2's coef is a half-swapped negative-stride view of c1)
                m1 = rtp.tile([P, HALF], BF16, tag="rt")
                m2 = rtp.tile([P, HALF], BF16, tag="rt")
                nc.vector.tensor_mul(m1, qsb, c1)
                c1v = c1.rearrange("p (h two i) -> p h two i",
                                   two=2, i=D2)[:, :, ::-1, :]
                q4 = qsb.rearrange("p (h two i) -> p h two i", two=2, i=D2)
                m2_4 = m2.rearrange("p (h two i) -> p h two i", two=2, i=D2)
                nc.vector.tensor_mul(m2_4, q4, c1v)
                pr3 = pr[:, hsl].rearrange("p (h d) -> p h d", d=D)
                m1_3 = m1.rearrange("p (h d) -> p h d", d=D)
                m2_3 = m2.rearrange("p (h d) -> p h d", d=D)
                # o_lo = x0*cos - x1*sin; o_hi = x1*cos + x0*sin
                nc.gpsimd.tensor_sub(pr3[:, :, 0:D2],
                                     m1_3[:, :, 0:D2], m1_3[:, :, D2:D])
                nc.gpsimd.tensor_add(pr3[:, :, D2:D],
                                     m2_3[:, :, 0:D2], m2_3[:, :, D2:D])

        for m in range(MCH):
            yield (lambda m=m: t_coef(m))
            for half in range(2):
                yield (lambda m=m, h=half: t_proj(m, h))

        def t_qkt(which, c):
            dstT = st["qT"] if which == "q" else st["kT"]
            ps = tpps.tile([P, S], BF16, tag="tp")
            for m in range(MCH):
                nc.tensor.transpose(ps[:, m * P:(m + 1) * P],
                                    st["prs"][which][m][:, c * P:(c + 1) * P],
                                    identb)
            nc.any.tensor_copy(dstT[:, c, :], ps)

        for c in range(CCH):
            yield (lambda c=c: t_qkt("q", c))
            yield (lambda c=c: t_qkt("k", c))

    def att_core_thunks(i, b):
        st = ST[i]

        def t_alloc():
            st["postxT"] = bigp.tile([P, CCH, S], BF16, tag="postxT",
                                     bufs=2, name=f"px_{i}")

        yield t_alloc

        e_tiles = {}

        def t_scores(hc):
            qT_sb, kT_sb, mb_t = st["qT"], st["kT"], st["mb_t"]
            # e tiles in fp8 jm-pairs [P, 2, S] for DoubleRow PV matmuls
            e_tiles[hc] = ([None, None], [None, None])
            for jm in range(MCH):
                for par in (0, 1):
                    h = 2 * hc + par
                    rows = slice(par * 64, par * 64 + 64)
                    ps_s = mmps.tile([P, S], F32, tag="mm",
                                     name=f"ss_{i}_{h}_{jm}")
                    nc.tensor.matmul(
                        ps_s,
                        lhsT=kT_sb[rows, hc, jm * P:(jm + 1) * P],
                        rhs=qT_sb[rows, hc, :],
                    )
                    if jm % 2 == 0:
                        e_tiles[hc][par][jm // 2] = etp.tile(
                            [P, 2, S], FP8, tag="et",
                            name=f"et_{i}_{h}_{jm // 2}")
                    e_t = e_tiles[hc][par][jm // 2]
                    nc.scalar.activation(e_t[:, jm % 2, :], ps_s, AF.Exp,
                                         bias=mb_t[jm], scale=1.0 / np.sqrt(D))

        def t_pv(hc):
            v_sb, postxT = st["v"], st["postxT"]
            for par in (0, 1):
                h = 2 * hc + par
                ps_pv = pvps.tile([P, S], F32, tag="pv", name=f"pv_{i}_{h}")
                for jp in range(MCH // 2):
                    nc.tensor.matmul(
                        ps_pv[0:65, :],
                        lhsT=v_sb[jp][:, :, h * 65:h * 65 + 65],
                        rhs=e_tiles[hc][par][jp][:],
                        start=(jp == 0),
                        stop=(jp == MCH // 2 - 1),
                        perf_mode=mybir.MatmulPerfMode.DoubleRow,
                    )
                rr_t = rsp.tile([1, S], F32, tag="rr")
                nc.vector.reciprocal(rr_t, ps_pv[64:65, :])
                rb_t = rbp.tile([64, S], F32, tag="rb")
                nc.gpsimd.partition_broadcast(rb_t, rr_t)
                dst = (postxT[0:64, hc, :] if par == 0
                       else postxT[64:128, hc, :])
                nc.vector.tensor_mul(dst, ps_pv[0:64, :], rb_t)
            del e_tiles[hc]

        # software pipeline: scores(hc+1) issues before pv(hc) so the PE
        # never waits on the exp chain of the head it just scored
        yield (lambda: t_scores(0))
        for hc in range(CCH):
            if hc + 1 < CCH:
                yield (lambda hc=hc: t_scores(hc + 1))
            yield (lambda hc=hc: t_pv(hc))

    def out_thunks(i, b, last=False):
        st = ST[i]

        def t_out_a(m):
            ps_o = mmps.tile([P, HID], F32, tag="mm")
            for c in range(CCH):
                nc.tensor.matmul(
                    ps_o,
                    lhsT=st["postxT"][:, c, m * P:(m + 1) * P],
                    rhs=woT[:, c, :],
                    start=(c == 0),
                    stop=False,
                )
            if not zb_o:
                nc.tensor.matmul(ps_o, lhsT=ones1r, rhs=bo_row,
                                 start=False, stop=False)
            # residual fold: ps_o += I.T @ xbf[m]  (adds x in one matmul)
            nc.tensor.matmul(ps_o, lhsT=identb, rhs=st["xbf"][m],
                             start=False, stop=True)
            y_t = lnp.tile([P, HID], F32, tag="y", bufs=4,
                           name=f"y_{i}_{m}")
            nc.vector.tensor_copy(y_t, ps_o)
            st_t = lnp.tile([P, 6], F32, tag="st")
            nc.vector.bn_stats(st_t, y_t)
            nc.vector.bn_aggr(st["mv4"][:, m, :], st_t)
            st["y_t"][m] = y_t

        def t_ln_scale(msl, part):
            # one Sqrt per batch (batched over the 4 s-chunks) so the ACT
            # table only round-trips exp->sqrt->exp once per batch element.
            # (the final batch splits in two so its tail overlaps better)
            mv4 = st["mv4"]
            nmch = msl.stop - msl.start
            sd4 = lnp.tile([P, nmch], F32, tag="sd4")
            nc.scalar.activation(sd4, mv4[:, msl, 1], AF.Sqrt, bias=eps_t)
            rstd4 = lnp.tile([P, nmch], F32, tag="rstd4", bufs=4,
                             name=f"rstd4_{i}_{part}")
            nc.vector.reciprocal(rstd4, sd4)
            # nmurs = -mu * rstd (per-partition bias for the affine tail)
            nmurs4 = lnp.tile([P, nmch], F32, tag="nmurs4", bufs=4,
                              name=f"nmurs4_{i}_{part}")
            nc.vector.scalar_tensor_tensor(nmurs4, mv4[:, msl, 0], -1.0,
                                           rstd4,
                                           op0=mybir.AluOpType.mult,
                                           op1=mybir.AluOpType.mult)
            for m in range(msl.start, msl.stop):
                st["rstd"][m] = rstd4[:, m - msl.start:m - msl.start + 1]
                st["nmurs"][m] = nmurs4[:, m - msl.start:m - msl.start + 1]

        def t_out_b(m):
            yb = yd[b].rearrange("(m p) h -> p m h", p=P)
            y_t = st["y_t"][m]
            y2 = lnp.tile([P, HID], F32, tag="y2", bufs=4)
            if ones_gamma:
                # y2 = (y_t * rstd + (-mu*rstd)) + 0
                nc.vector.affine_then_add(y2, y_t, zeros_sb,
                                          scale=st["rstd"][m],
                                          bias=st["nmurs"][m])
            else:
                nc.vector.scalar_tensor_tensor(y2, y_t, st["mv4"][:, m, 0:1],
                                               gamma_sb,
                                               op0=mybir.AluOpType.subtract,
                                               op1=mybir.AluOpType.mult)
                nc.vector.tensor_scalar_mul(y2, y2, st["rstd"][m])
            if not z_beta:
                nc.gpsimd.tensor_add(y2, y2, beta_sb)
            nc.sync.dma_start(yb[:, m, :], y2)

        def t_alloc_mv():
            st["mv4"] = lnp.tile([P, MCH, 2], F32, tag="mv4", bufs=2,
                                 name=f"mv4_{i}")
            st["y_t"] = [None] * MCH
            st["rstd"] = [None] * MCH
            st["nmurs"] = [None] * MCH

        yield t_alloc_mv
        if last:
            # per-m LN groups: out_a(m+1) overlaps affine(m)+DMA(m); no
            # extra table loads since no exps follow the final batch
            for m in range(MCH):
                yield (lambda m=m: t_out_a(m))
                yield (lambda m=m: t_ln_scale(slice(m, m + 1), m))
                yield (lambda m=m: t_out_b(m))
        else:
            for m in range(MCH):
                yield (lambda m=m: t_out_a(m))
            yield (lambda: t_ln_scale(slice(0, MCH), 0))
            for m in range(MCH):
                yield (lambda m=m: t_out_b(m))

    # ---- prologue: ident first (tiny, gates transposes), then batch-0
    # xbf/masks, then weights (first proj waits on them) ----
    identb = consts.tile([P, P], BF16, tag="identb")
    nc.sync.dma_start(identb, identd[:])
    pre0 = list(prep_thunks(0, batches[0]))
    pre0[0]()                                   # xbf/mask DMAs
    wqT = consts.tile([P, KCH, NHD], BF16, tag="wqT")
    nc.sync.dma_start(wqT, wqtd[:])
    wkT = consts.tile([P, KCH, NHD], BF16, tag="wkT")
    nc.sync.dma_start(wkT, wktd[:])
    wvT = consts.tile([P, KCH, NHD], BF16, tag="wvT")
    nc.sync.dma_start(wvT, wvtd[:])
    if not zb_qkv:
        bq_sb = consts.tile([1, NHD], BF16, tag="bq")
        nc.sync.dma_start(bq_sb, bqd[:])
        bk_sb = consts.tile([1, NHD], BF16, tag="bk")
        nc.sync.dma_start(bk_sb, bkd[:])
        bv_sb = consts.tile([1, NHD], BF16, tag="bv")
        nc.sync.dma_start(bv_sb, bvd[:])
        ones1 = consts.tile([1, P], BF16, tag="ones1")
        nc.vector.memset(ones1, 1.0)
    if not zb_o:
        bo_row = consts.tile([1, HID], F32R, tag="bo_row")
        nc.sync.dma_start(bo_row, bod[:])
        ones1r = consts.tile([1, P], F32R, tag="ones1r")
        nc.vector.memset(ones1r.bitcast(F32), 1.0)
    eps_t = consts.tile([P, 1], F32, tag="eps")
    nc.vector.memset(eps_t, LN_EPS)
    if ones_gamma:
        zeros_sb = consts.tile([P, HID], F32, tag="zeros")
        nc.vector.memset(zeros_sb, 0.0)
    for t in pre0[1:]:
        t()
    woT = consts.tile([P, CCH, HID], BF16, tag="woT")
    nc.sync.dma_start(woT, wotd[:])
    if not ones_gamma:
        gamma_sb = consts.tile([P, HID], F32, tag="gamma")
        nc.sync.dma_start(gamma_sb, gammad[:].to_broadcast([P, HID]))
    if not z_beta:
        beta_sb = consts.tile([P, HID], F32, tag="beta")
        nc.sync.dma_start(beta_sb, betad[:].to_broadcast([P, HID]))

    # ---- steady state ----
    # batch i's attention runs interleaved with batch i+1's prep AND
    # batch i-1's output/LN phase (fills PE bubbles during exp waits)
    def merge(a_list, b_list):
        """Proportionally interleave b_list into a_list."""
        na, nn = len(a_list), len(b_list)
        out = []
        ai = ni = 0
        while ai < na or ni < nn:
            if ai < na:
                out.append(a_list[ai])
                ai += 1
            take = (nn * (ai + 1)) // max(na, 1) - ni if na else nn
            for _ in range(max(0, take)):
                if ni < nn:
                    out.append(b_list[ni])
                    ni += 1
        return out

    prev_out = []
    for i, b in enumerate(batches):
        attc = list(att_core_thunks(i, b))
        stream_a = merge(attc, prev_out)
        nxt = (list(prep_thunks(i + 1, batches[i + 1]))
               if i + 1 < len(batches) else [])
        for t in merge(stream_a, nxt):
            t()
        prev_out = list(out_thunks(i, b, last=(i + 1 == len(batches))))
    for t in prev_out:
        t()


_NC = {}


def build(repeat=1, flags=(True, True, True, True)):
    key = (repeat, flags)
    if key in _NC:
        return _NC[key]
    nc = bacc.Bacc("TRN2", target_bir_lowering=False, debug=False,
                   enable_asserts=False, num_devices=NCORES)
    from contextlib import ExitStack
    with tile.TileContext(nc) as tc, ExitStack() as ctx:
        _emit(ctx, tc, repeat=repeat, flags=flags)
    nc.compile()
    _NC[key] = nc
    return nc


_PERM = np.concatenate([np.arange(0, D, 2), np.arange(1, D, 2)])  # evens|odds
_COLPERM = (np.arange(NH)[:, None] * D + _PERM[None, :]).reshape(-1)


def input_flags(inputs):
    zb_qkv = not (np.any(inputs["bq"]) or np.any(inputs["bk"])
                  or np.any(inputs["bv"]))
    zb_o = not np.any(inputs["bo"])
    z_beta = not np.any(inputs["ln_beta"])
    ones_gamma = bool(np.all(np.asarray(inputs["ln_gamma"]) == 1.0))
    return (zb_qkv, zb_o, z_beta, ones_gamma)


def make_in_maps(inputs):
    x = np.ascontiguousarray(np.asarray(inputs["x"], dtype=np.float32))
    # rope coefs: input layout per head is [sin(32) | cos(32)].
    # duplicate each half -> [cos|cos], [sin|sin]
    def coefs(name):
        # input layout per head: [sin(32) | cos(32)]; device expects
        # [cos | sin] (second rope mul reads it half-swapped on device)
        c = np.asarray(inputs[name], np.float32)      # [B, S, NH, D]
        sin, cos = c[..., :D2], c[..., D2:]
        c1 = np.concatenate([cos, sin], axis=-1).reshape(B, S, NHD)
        return np.ascontiguousarray(c1.astype(ml_dtypes.bfloat16))
    cq1 = coefs("product_ion_info_query")
    ck1 = coefs("product_ion_info_key")
    mask = np.asarray(inputs["src_key_padding_mask"]).reshape(B, S)
    maskb = np.where(mask, 0.0, MASK_NEG).astype(np.float32).reshape(B, S, 1)
    def tile_w(w, nch):
        # [nch*P, free] -> [P, nch, free] (partition-contiguous DMA)
        return np.ascontiguousarray(
            w.reshape(nch, P, w.shape[-1]).transpose(1, 0, 2))

    def tile_w_half(w):
        # [KCH*P, NHD] -> [2, P, KCH, HALF], half-major contiguous
        t = tile_w(w, KCH)                        # [P, KCH, NHD]
        return np.ascontiguousarray(
            t.reshape(P, KCH, 2, HALF).transpose(2, 0, 1, 3))
    # permute q/k weight rows per head to [even components | odd components]
    wqT = tile_w(np.asarray(inputs["Wq"], np.float32).T[:, _COLPERM]
                 .astype(ml_dtypes.bfloat16), KCH)
    wkT = tile_w(np.asarray(inputs["Wk"], np.float32).T[:, _COLPERM]
                 .astype(ml_dtypes.bfloat16), KCH)
    wvT = tile_w(np.asarray(inputs["Wv"], np.float32).T
                 .astype(ml_dtypes.bfloat16), KCH)
    woT = tile_w(np.asarray(inputs["Wo"], np.float32).T
                 .astype(ml_dtypes.bfloat16), CCH)
    bq = np.asarray(inputs["bq"], np.float32)[_COLPERM].reshape(1, NHD)
    bk = np.asarray(inputs["bk"], np.float32)[_COLPERM].reshape(1, NHD)
    shared = dict(
        wqT=wqT, wkT=wkT, wvT=wvT, woT=woT,
        bq=bq.astype(ml_dtypes.bfloat16),
        bk=bk.astype(ml_dtypes.bfloat16),
        bv=np.asarray(inputs["bv"], np.float32).reshape(1, NHD).astype(ml_dtypes.bfloat16),
        bo=np.asarray(inputs["bo"], np.float32).reshape(1, HID),
        gamma=np.asarray(inputs["ln_gamma"], np.float32).reshape(1, HID),
        beta=np.asarray(inputs["ln_beta"], np.float32).reshape(1, HID),
        ident=np.eye(P, dtype=ml_dtypes.bfloat16),
    )
    # pre-tile x to [B, P, MCH, HID]; only the bf16 copy goes to device
    xbf16 = np.ascontiguousarray(
        x.reshape(B, MCH, P, HID).transpose(0, 2, 1, 3)
        .astype(ml_dtypes.bfloat16))
    in_maps = []
    for c in range(NCORES):
        sl = slice(c * BPC, (c + 1) * BPC)
        in_maps.append(dict(
            xbf16=np.ascontiguousarray(xbf16[sl]),
            cq1=np.ascontiguousarray(cq1[sl]),
            ck1=np.ascontiguousarray(ck1[sl]),
            maskb=np.ascontiguousarray(maskb[sl]),
            **shared,
        ))
    return in_maps


LAST_RESULTS = None
_RUNNERS = {}


def kernel(_repeat=1, **inputs):
    global LAST_RESULTS
    flags = input_flags(inputs)
    key = (_repeat, flags)
    if key not in _RUNNERS:
        _RUNNERS[key] = make_runner(_repeat, flags)
    in_maps = make_in_maps(inputs)
    out_arrs = _RUNNERS[key](in_maps)
    out = np.asarray(out_arrs[0]).reshape(NCORES, BPC, S, HID)
    return out.reshape(B, S, HID).astype(np.float32)


def make_runner(repeat=1, flags=(True, True, True, True)):
    """Build the sharded 8-core jit once; return f(in_maps)->list per-core outs.

    Mirrors bass2jax.run_bass_via_pjrt's multi-core path, but reusable so
    repeated calls skip retrace/recompile (for timing)."""
    import jax
    from jax.experimental.shard_map import shard_map
    from jax.sharding import Mesh, PartitionSpec
    from concourse import bass2jax

    nc = build(repeat, flags)
    bass2jax.install_neuronx_cc_hook()
    partition_name = (nc.partition_id_tensor.name
                      if nc.partition_id_tensor else None)
    in_names, out_names, out_avals, zero_outs = [], [], [], []
    for alloc in nc.m.functions[0].allocations:
        if not isinstance(alloc, mybir.MemoryLocationSet):
            continue
        name = alloc.memorylocations[0].name
        if alloc.kind == "ExternalInput":
            if name != partition_name:
                in_names.append(name)
        elif alloc.kind == "ExternalOutput":
            shape = tuple(alloc.tensor_shape)
            dtype = mybir.dt.np(alloc.dtype)
            out_names.append(name)
            out_avals.append(jax.core.ShapedArray(shape, dtype))
            zero_outs.append(np.zeros(shape, dtype))
    n_params = len(in_names)
    all_in_names = list(in_names) + list(out_names)
    if partition_name is not None:
        all_in_names.append(partition_name)

    def _body(*args):
        operands = list(args)
        if partition_name is not None:
            operands.append(bass2jax.partition_id_tensor())
        outs = bass2jax._bass_exec_p.bind(
            *operands,
            out_avals=tuple(out_avals),
            in_names=tuple(all_in_names),
            out_names=tuple(out_names),
            lowering_input_output_aliases=(),
            sim_require_finite=True,
            sim_require_nnan=True,
            nc=nc,
        )
        return tuple(outs)

    devices = jax.devices()[:NCORES]
    mesh = Mesh(np.asarray(devices), ("core",))
    n_outs = len(out_names)
    sharded = jax.jit(
        shard_map(_body, mesh=mesh,
                  in_specs=(PartitionSpec("core"),) * (n_params + n_outs),
                  out_specs=(PartitionSpec("core"),) * n_outs,
                  check_rep=False),
        keep_unused=True,
    )
    concat_zeros = [np.zeros((NCORES * z.shape[0], *z.shape[1:]), z.dtype)
                    for z in zero_outs]

    def run(in_maps):
        per_core = [[np.asarray(m[n]) for n in in_names] for m in in_maps]
        concat_in = [np.concatenate([per_core[c][i] for c in range(NCORES)],
                                    axis=0) for i in range(n_params)]
        out_arrs = sharded(*concat_in, *concat_zeros)
        jax.block_until_ready(out_arrs)
        return out_arrs

    return run
